# revision 1
# baseline (speedup 1.0000x reference)
"""Max-unpool (DePooling2D) Trainium2 kernel.

Full inputs: net [8,56,56,256] f32, mask [8,56,56,256] int64 (tf argmax
encoding ((y*oW)+x)*C + c with y=2h+dy, x=2w+dx, dy,dx in {0,1}), stride=2.
Output: [8,112,112,256] f32 with net scattered to (2h+dy, 2w+dx, c), zeros
elsewhere.

Strategy (one image per NeuronCore, batch sharded across the 8 cores):

- Only a 5-bit window of the mask is information-bearing:
  m5 = (mask >> 8) & 31 = (2w + 16dy + dx) mod 32  (the 224h term is
  0 mod 32, c < 256 never carries).  The host ships m5 as f16; net is
  shipped f16 and the output produced f16 (rel err ~2e-4, upcast on host).
- w-major layout: partition p = hh*56 + w (h = 28*hh + hl), so the known
  part of m5 is PER-PARTITION.  The entire decode collapses into the four
  select ops: out_ij = (m5 == (2w + 16i + j) mod 32) * net, each a single
  fused DVE scalar_tensor_tensor with a per-partition scalar pointer --
  no subtract/mod chain at all.  DVE work is exactly the output-write
  floor (4 x 1792 elems per group).
- hl is tiled in 4 groups of 7 rows; per group 4 input DMAs (net/m5 per
  h-half) and 4 output DMAs (hh,i) alternate the two HWDGE rings.  The out
  tile keeps (j c) innermost so the two x-columns of each w merge into
  1024B HBM runs (512B for inputs) -- measured at the DMA roofline.
"""

import numpy as np

import concourse.bass as bass
import concourse.mybir as mybir
from concourse import bacc, bass_utils
from concourse.tile import TileContext

B, H, W, C = 8, 56, 56, 256
OH, OW = 2 * H, 2 * W
HH, HL2, HL = 2, 4, 7

_FP = mybir.dt.float32
_F16 = mybir.dt.float16
_I32 = mybir.dt.int32


def _build_bass(nrep: int = 1, loop_n: int = 0) -> bass.Bass:
    """nrep>1 statically repeats the pass inside one NEFF body; loop_n=-1
    wraps it in a hardware For_i whose trip count is a runtime input (both
    benchmarking-only)."""
    nc = bacc.Bacc("TRN2", target_bir_lowering=False, debug=False)
    net = nc.dram_tensor("net", [H, W, C], _F16, kind="ExternalInput").ap()
    m5 = nc.dram_tensor("m5", [H, W, C], _F16, kind="ExternalInput").ap()
    scmp = nc.dram_tensor("scmp", [112, 4], _F16, kind="ExternalInput").ap()
    out = nc.dram_tensor("out", [OH, OW, C], _F16, kind="ExternalOutput").ap()
    bench = loop_n != 0 or nrep > 1
    done = nc.dram_tensor("done", [1, 64], _FP, kind="ExternalOutput").ap() if bench else None
    tok = nc.dram_tensor("tok", [1, 64], _FP, kind="ExternalInput").ap() if bench else None
    nloop = (
        nc.dram_tensor("nloop", [1, 1], _I32, kind="ExternalInput").ap()
        if loop_n == -1
        else None
    )

    net_r = net.rearrange("(hh hl2 hl) w c -> hh hl2 w hl c", hh=HH, hl2=HL2, hl=HL)
    m5_r = m5.rearrange("(hh hl2 hl) w c -> hh hl2 w hl c", hh=HH, hl2=HL2, hl=HL)
    # y = 56*hh + 14*g + 2*hl + i ; x = 2*w + j.  For fixed w the two j
    # columns are adjacent in HBM, so (j c) merges into 1024B runs.
    out_r = out.rearrange(
        "(hh hl2 hl i) (w j) c -> hh hl2 i w hl (j c)",
        hh=HH, hl2=HL2, hl=HL, i=2, w=56, j=2,
    )

    with TileContext(nc) as tc:
        with (
            tc.tile_pool(name="cst", bufs=1) as cst,
            tc.tile_pool(name="netp", bufs=4) as netp,
            tc.tile_pool(name="m5p", bufs=4) as m5p,
            tc.tile_pool(name="outp", bufs=4) as outp,
        ):
            scmpt = cst.tile([112, 4], _F16)
            nc.sync.dma_start(out=scmpt[:], in_=scmp)

            def _group(g):
                nett = netp.tile([112, HL, C], _F16, tag="net")
                m5t = m5p.tile([112, HL, C], _F16, tag="m5")
                outt = outp.tile([112, 2, HL, 2, C], _F16, tag="out")
                for hh in range(2):
                    sl = slice(hh * 56, (hh + 1) * 56)
                    eng_n = (nc.sync, nc.scalar)[hh]
                    eng_m = (nc.scalar, nc.sync)[hh]
                    eng_n.dma_start(out=nett[sl], in_=net_r[hh, g])
                    eng_m.dma_start(out=m5t[sl], in_=m5_r[hh, g])
                for i in range(2):
                    for j in range(2):
                        nc.vector.scalar_tensor_tensor(
                            out=outt[:, i, :, j, :],
                            in0=m5t[:],
                            scalar=scmpt[:, 2 * i + j : 2 * i + j + 1],
                            in1=nett[:],
                            op0=mybir.AluOpType.is_equal,
                            op1=mybir.AluOpType.mult,
                        )
                for i in range(2):
                    for hh in range(2):
                        eng = (nc.sync, nc.scalar)[(i + hh) % 2]
                        eng.dma_start(
                            out=out_r[hh, g, i],
                            in_=outt[hh * 56 : (hh + 1) * 56, i].rearrange(
                                "p a j c -> p a (j c)"
                            ),
                        )

            def _pass():
                for g in range(HL2):
                    _group(g)

            if loop_n == -1:
                nloopt = cst.tile([1, 1], _I32)
                nc.sync.dma_start(out=nloopt[:], in_=nloop)
                nv = nc.values_load(
                    nloopt[0:1, 0:1], min_val=0, max_val=1 << 20,
                    skip_runtime_bounds_check=True,
                )
                with tc.For_i(0, nv, 1):
                    for _ in range(nrep):
                        _pass()
            elif loop_n > 0:
                with tc.For_i(0, loop_n, 1):
                    for _ in range(nrep):
                        _pass()
            else:
                for _ in range(nrep):
                    _pass()
            if done is not None:
                tokt = cst.tile([1, 64], _FP)
                nc.sync.dma_start(out=tokt[:], in_=tok)
                nc.sync.dma_start(out=done, in_=tokt[:])
    nc.compile()
    return nc


def _make_scmp() -> np.ndarray:
    # scmp[p, 2i+j] = (2*(p mod 56) + 16i + j) mod 32
    p = np.arange(112)
    w = p % 56
    v = np.zeros((112, 4), np.int64)
    for i in range(2):
        for j in range(2):
            v[:, 2 * i + j] = (2 * w + 16 * i + j) % 32
    return v.astype(np.float16)


def _mask_m5(mask: np.ndarray) -> np.ndarray:
    """m5 = (mask >> 8) & 31 as f16, via the byte-1 view (no wide math)."""
    if mask.dtype in (np.int64, np.uint64):
        b1 = mask.view(np.uint8)[..., 1::8]
    elif mask.dtype in (np.int32, np.uint32):
        b1 = mask.view(np.uint8)[..., 1::4]
    else:
        b1 = ((np.ascontiguousarray(mask).astype(np.int64) >> 8) & 0xFF).astype(
            np.uint8
        )
    return (b1 & 31).astype(np.float16)


_NC_CACHE: dict[tuple, bass.Bass] = {}


def _get_nc(nrep: int = 1, loop_n: int = 0) -> bass.Bass:
    key = (nrep, loop_n)
    if key not in _NC_CACHE:
        _NC_CACHE[key] = _build_bass(nrep, loop_n)
    return _NC_CACHE[key]


def kernel(net: np.ndarray, mask: np.ndarray, stride=None, **run_kwargs):
    net = np.asarray(net)
    mask = np.asarray(mask)
    assert net.shape == (B, H, W, C) and mask.shape == (B, H, W, C)
    net16 = np.ascontiguousarray(net, dtype=np.float32).astype(np.float16)
    m5 = _mask_m5(mask).reshape(B, H, W, C)
    scmp = _make_scmp()
    in_maps = [
        {"net": net16[k], "m5": m5[k], "scmp": scmp} for k in range(B)
    ]
    nc = _get_nc()
    res = bass_utils.run_bass_kernel_spmd(nc, in_maps, list(range(B)), **run_kwargs)
    out = np.stack([res.results[k]["out"] for k in range(B)], axis=0)
    if run_kwargs:
        kernel.last_results = res
    return out.astype(np.float32)



# revision 7
# speedup vs baseline: 1.7427x; 1.7427x over previous
"""Max-unpool (DePooling2D) Trainium2 kernel.

Full inputs: net [8,56,56,256] f32, mask [8,56,56,256] int64 (tf argmax
encoding ((y*oW)+x)*C + c with y=2h+dy, x=2w+dx, dy,dx in {0,1}), stride=2.
Output: [8,112,112,256] f32 with net scattered to (2h+dy, 2w+dx, c), zeros
elsewhere.

Strategy (one image per NeuronCore, batch sharded across the 8 cores):

- Partition p = 2h + s where s = w-half (w = 28 s + wl).  Each partition
  owns the two output rows oh = 2h+{0,1} over its half-row ow in
  [56 s, 56 s + 56).  The output DMA for select-plane i then writes one
  14336 B contiguous HBM run per partition per w-group -- ~14x longer
  runs than a w-partitioned layout (descriptor count ~0.9k vs ~12.5k
  per pass).
- Only dy,dx matter: host ships d2 = 2*dy+dx as 2-bit fields packed 8-up
  in uint16 words (0.2 MB/core vs 1.6 MB for a f16 plane).  Device
  decode is 8 dual-op tensor_scalar (>>2k & 3, u16 -> u16) per group,
  running in the 4x_2p DVE mode (0.25 cyc/elem).
- Select out_ij = (d2 == 2i+j) * net is split across engines so the DVE
  (0.96 GHz, and scalar_tensor_tensor has no fast mode) stays off the
  critical path: plane i=0 on DVE as indicator (tensor_scalar is_equal,
  4x mode) + tensor_tensor mult (2x mode); plane i=1 on GpSimd as fused
  scalar_tensor_tensor.
- Traffic/core/pass: net 1.6 MB + packed mask 0.2 MB + out 6.4 MB
  = 8.23 MB -> ~23 us at the 360 GB/s DMA roofline.
"""

import numpy as np

import concourse.bass as bass
import concourse.mybir as mybir
from concourse import bacc, bass_utils
from concourse.tile import TileContext

B, H, W, C = 8, 56, 56, 256
OH, OW = 2 * H, 2 * W
WG, WL = 2, 14  # split the 28 w-per-half into WG groups of WL

_FP = mybir.dt.float32
_F16 = mybir.dt.float16
_U16 = mybir.dt.uint16
_I32 = mybir.dt.int32

# gpsimd cannot run TensorScalarPtr (walrus: "Instruction engine check
# failed (Pool)") -- everything elementwise runs on the DVE.
USE_GPSIMD = False


def _build_bass(nrep: int = 1, loop_n: int = 0) -> bass.Bass:
    """nrep>1 statically repeats the pass inside one NEFF body; loop_n=-1
    wraps it in a hardware For_i whose trip count is a runtime input (both
    benchmarking-only)."""
    nc = bacc.Bacc("TRN2", target_bir_lowering=False, debug=False)
    net = nc.dram_tensor("net", [112, 28, C], _F16, kind="ExternalInput").ap()
    w16 = nc.dram_tensor("w16", [112, 28, 32], _U16, kind="ExternalInput").ap()
    out = nc.dram_tensor("out", [OH, OW * C], _F16, kind="ExternalOutput").ap()
    bench = loop_n != 0 or nrep > 1
    done = nc.dram_tensor("done", [1, 64], _FP, kind="ExternalOutput").ap() if bench else None
    tok = nc.dram_tensor("tok", [1, 64], _FP, kind="ExternalInput").ap() if bench else None
    nloop = (
        nc.dram_tensor("nloop", [1, 1], _I32, kind="ExternalInput").ap()
        if loop_n == -1
        else None
    )

    net_r = net.rearrange("p (wg wl) c -> wg p (wl c)", wg=WG, wl=WL)
    w16_r = w16.rearrange("p (wg wl) t -> wg p (wl t)", wg=WG, wl=WL)
    # oh = 2h+i, ow = 56 s + 2 (14 wg + wl) + j; partition p = 2h+s.
    # (h s) can't merge into one AP dim (offset not linear in p=2h+s); keep
    # them separate -- the DMA pairs SBUF [112, x] with DRAM [56, 2, x] in
    # linear iteration order, which is exactly p = 2h+s.
    out_r = out.rearrange(
        "(h i) (s wg wl j c) -> i wg h s (wl j c)",
        h=56, i=2, s=2, wg=WG, wl=WL, j=2, c=C,
    )

    with TileContext(nc) as tc:
        with (
            tc.tile_pool(name="cst", bufs=1) as cst,
            tc.tile_pool(name="netp", bufs=3) as netp,
            tc.tile_pool(name="w16p", bufs=3) as w16p,
            tc.tile_pool(name="d2ip", bufs=3) as d2ip,
            tc.tile_pool(name="indp", bufs=3) as indp,
            tc.tile_pool(name="outp", bufs=3) as outp,
        ):
            def _group(g):
                nett = netp.tile([112, WL, C], _F16, tag="net")
                w16t = w16p.tile([112, WL, 32], _U16, tag="w16")
                d2it = d2ip.tile([112, WL, C], _U16, tag="d2i")
                outt = outp.tile([112, 2, WL, 2, C], _F16, tag="out")
                eng_n = (nc.sync, nc.scalar)[g % 2]
                eng_m = (nc.scalar, nc.sync)[g % 2]
                eng_n.dma_start(
                    out=nett[:].rearrange("p wl c -> p (wl c)"), in_=net_r[g]
                )
                eng_m.dma_start(
                    out=w16t[:].rearrange("p wl t -> p (wl t)"), in_=w16_r[g]
                )
                for k in range(8):
                    nc.vector.tensor_scalar(
                        out=d2it[:, :, 32 * k : 32 * k + 32],
                        in0=w16t[:],
                        scalar1=2 * k,
                        scalar2=3,
                        op0=mybir.AluOpType.logical_shift_right,
                        op1=mybir.AluOpType.bitwise_and,
                    )
                # selects: indicator (tensor_scalar is_equal, 4x mode) then
                # tensor_tensor mult (2x mode) -- 0.75 cyc/out-elem, vs 1.0
                # for the fused scalar_tensor_tensor (which has no fast mode).
                for i in range(2):
                    for j in range(2):
                        indt = indp.tile([112, WL, C], _F16, tag="ind")
                        nc.vector.tensor_scalar(
                            out=indt[:],
                            in0=d2it[:],
                            scalar1=float(2 * i + j),
                            scalar2=None,
                            op0=mybir.AluOpType.is_equal,
                        )
                        nc.vector.tensor_tensor(
                            out=outt[:, i, :, j, :],
                            in0=indt[:],
                            in1=nett[:],
                            op=mybir.AluOpType.mult,
                        )
                for i in range(2):
                    eng = (nc.sync, nc.scalar)[(g + i) % 2]
                    eng.dma_start(
                        out=out_r[i, g],
                        in_=outt[:, i].rearrange("p wl j c -> p (wl j c)"),
                    )

            def _pass():
                for g in range(WG):
                    _group(g)

            if loop_n == -1:
                nloopt = cst.tile([1, 1], _I32)
                nc.sync.dma_start(out=nloopt[:], in_=nloop)
                nv = nc.values_load(
                    nloopt[0:1, 0:1], min_val=0, max_val=1 << 20,
                    skip_runtime_bounds_check=True,
                )
                with tc.For_i(0, nv, 1):
                    for _ in range(nrep):
                        _pass()
            elif loop_n > 0:
                with tc.For_i(0, loop_n, 1):
                    for _ in range(nrep):
                        _pass()
            else:
                for _ in range(nrep):
                    _pass()
            if done is not None:
                tokt = cst.tile([1, 64], _FP)
                nc.sync.dma_start(out=tokt[:], in_=tok)
                nc.sync.dma_start(out=done, in_=tokt[:])
    nc.compile()
    return nc


def make_device_maps(net: np.ndarray, mask: np.ndarray) -> list[dict]:
    """Per-core device input maps (core b gets image b)."""
    net = np.asarray(net)
    mask = np.asarray(mask)
    assert net.shape == (B, H, W, C) and mask.shape == (B, H, W, C)
    net16 = np.ascontiguousarray(net, dtype=np.float32).astype(np.float16)
    net16 = net16.reshape(B, 112, 28, C)
    t = (np.ascontiguousarray(mask).astype(np.uint32)) >> 8  # 224h+112dy+2w+dx
    d2 = ((((t // 112) & 1) << 1) | (t & 1)).astype(np.uint16)
    d2 = d2.reshape(B, 112, 28, 8, 32)
    w16 = np.zeros((B, 112, 28, 32), np.uint16)
    for k in range(8):
        w16 |= d2[:, :, :, k, :] << (2 * k)
    return [{"net": net16[b], "w16": w16[b]} for b in range(B)]


_NC_CACHE: dict[tuple, bass.Bass] = {}


def _get_nc(nrep: int = 1, loop_n: int = 0) -> bass.Bass:
    key = (nrep, loop_n)
    if key not in _NC_CACHE:
        _NC_CACHE[key] = _build_bass(nrep, loop_n)
    return _NC_CACHE[key]


def kernel(net: np.ndarray, mask: np.ndarray, stride=None, **run_kwargs):
    in_maps = make_device_maps(net, mask)
    nc = _get_nc()
    res = bass_utils.run_bass_kernel_spmd(nc, in_maps, list(range(B)), **run_kwargs)
    out = np.stack(
        [res.results[k]["out"].reshape(OH, OW, C) for k in range(B)], axis=0
    )
    if run_kwargs:
        kernel.last_results = res
    return out.astype(np.float32)
